# revision 20
# baseline (speedup 1.0000x reference)
"""Trainium2 Bass kernel for nn_ELiCiT_50087908606687 (vq_codebook), v2.

Math (forward only): with X = p0 + S0·rsq0 (mode-0 node table),
Y = p1 + S1·rsq1 (mode-1), S0[d] = sum_{e:i0=d} (ipy[i1[e]]-0.5), the
reference einsum collapses per edge n to:

    out[n] = sum_f (A_f X[i0,f] + C_f) Y[i1,f] + (B·X[i0] + sumD)

with A = s(V0-V1-V2+V3), B = s(V1-V3), C = s(V2-V3), sumD = s·sum(V3)
(scale s folded in). The -0.5 offsets are baked into the stored ip
tables; rsq/count metadata is host-derived from idxs alone.

Device decomposition (8 cores, 3 SPMD launches, dest-range sharding):
  K1: quantize, features-on-partition: per boundary one fused
      tensor_scalar (is_gt ×delta, 4x DVE mode) + one f16 add; PE
      transposes back to row-major [px|ipx], [py|ipy] tables.
  KB: per core (i1 ranges): dma_gather [px|ipx] rows at i0 (one 256B/edge
      stream), one-hot matmuls accumulate S1 in PSUM per 128-row window,
      flush Y = py + rsq1·S1.
  KC: per core (i0 ranges): gather [ipy|Y] rows at i1; scatter matmuls
      (host-built one-hot ind) → S0 → X → XA65=[A∘X+C | B·X+sumD];
      selection matmuls (host-built ind_t) pick XA65 rows per edge;
      one tensor_tensor_reduce per chunk computes the 64-dot + r
      against the kept gather tiles' Y half.
Host does sharding, sorting/packing of index streams, one-hot masks,
bincounts, and inter-launch assembly only.
"""
import sys

sys.path.insert(0, "/opt/trn_rl_repo")

import os

import numpy as np
import ml_dtypes

try:
    import prof_shim  # noqa: F401  (registers NTFF hook when available)
except Exception:
    pass

import concourse.bacc as bacc
import concourse.mybir as mybir
import concourse.tile as tile
from concourse.bass_utils import run_bass_kernel_spmd
from concourse.masks import make_identity

TRACE = bool(int(os.environ.get("KERNEL_TRACE", "0")))
LAST_HW_NS = []

NC = 8
D0 = 50000
P = 128
F = 64
NQ = 16
NB = 15
R = 6272          # dest rows per core (49 windows of 128)
W = 49
HALF = 32768      # int16 split point for full-range gathers
NRY = NC * R      # padded full-table rows (50176)
CHUNK = 128
CALL = 8          # chunks per dma_gather call (1024 idxs)

f32 = mybir.dt.float32
bf16 = mybir.dt.bfloat16
f16 = mybir.dt.float16
i16 = mybir.dt.int16
BF = ml_dtypes.bfloat16
F8 = ml_dtypes.float8_e4m3
fp8 = mybir.dt.float8e4


def _run(nc, maps):
    r = run_bass_kernel_spmd(nc, maps, list(range(NC)), trace=TRACE)
    if TRACE:
        LAST_HW_NS.append(r.exec_time_ns or 0)
    return r.results


# ---------------------------------------------------------------- host utils
def _wrap16(flat):
    """Pack flat idx list (multiple of 1024) into (128, n*64) int16 SWDGE
    layout: per 1024-call, j -> [j % 16, j // 16], replicated 8x down."""
    ncall = len(flat) // 1024
    cols = []
    for c in range(ncall):
        a = flat[c * 1024:(c + 1) * 1024].reshape(64, 16).T  # (16, 64)
        cols.append(np.tile(a, (8, 1)))                      # (128, 64)
    return np.concatenate(cols, axis=1).astype(np.int16)     # (128, ncall*64)


def _pack_pass(dst, src, want_sel):
    """Bucket edges by dest core/window, order low/high by src-half, pad.

    Returns per-core dicts: gl/gh (wrap16 idx streams), ind (P, C*128)
    one-hot [edge-partition, destrow-free] bf16, ind_t transposed, perm,
    plus meta with per-window chunk counts and window→chunk spans.
    """
    core = np.minimum(dst // R, NC - 1)
    loc = dst - core * R
    w = loc // CHUNK
    rd = loc % CHUNK
    hi = (src >= HALF).astype(np.int8)

    order = np.lexsort((hi, w, core))
    oc, ow, ohi = core[order], w[order], hi[order]
    ord_src, ord_rd = src[order], rd[order]

    cnt = np.zeros((NC, W, 2), np.int64)
    np.add.at(cnt, (oc, ow, ohi), 1)
    KL = np.maximum(np.ceil(cnt[:, :, 0] / CHUNK).astype(np.int64).max(axis=0), 1)
    KH = np.maximum(np.ceil(cnt[:, :, 1] / CHUNK).astype(np.int64).max(axis=0), 1)

    C_total = int((KL + KH).sum())
    nlow = int(KL.sum())
    nhigh = int(KH.sum())
    ncall_low = -(-nlow // CALL)
    ncall_high = -(-nhigh // CALL)

    # stream position of each (window, chunk-within-half)
    low_pos = {}
    high_pos = {}
    pl = ph = 0
    for wi in range(W):
        for c in range(int(KL[wi])):
            low_pos[(wi, c)] = pl
            pl += 1
        for c in range(int(KH[wi])):
            high_pos[(wi, c)] = ph
            ph += 1

    cores = []
    core_starts = np.searchsorted(oc, np.arange(NC + 1))
    for k in range(NC):
        s, e = core_starts[k], core_starts[k + 1]
        cw, chi = ow[s:e], ohi[s:e]
        csrc, crd, cord = ord_src[s:e], ord_rd[s:e], order[s:e]

        gl = np.zeros(ncall_low * 1024, np.int64)
        gh = np.zeros(ncall_high * 1024, np.int64)
        # flat (ci, p, rdval) triples for one-hot builds
        tri_ci = np.empty(e - s, np.int64)
        tri_p = np.empty(e - s, np.int64)
        perm = np.full((C_total, P), -1, np.int64) if want_sel else None

        key = cw * 2 + chi
        starts = np.searchsorted(key, np.arange(2 * W + 1))
        ci = 0
        for wi in range(W):
            for half, K_half, posmap, g in ((0, int(KL[wi]), low_pos, gl),
                                            (1, int(KH[wi]), high_pos, gh)):
                a, b = starts[wi * 2 + half], starts[wi * 2 + half + 1]
                n = b - a
                for c in range(K_half):
                    lo = a + c * CHUNK
                    m = min(CHUNK, max(0, n - c * CHUNK))
                    pos = posmap[(wi, c)]
                    if m > 0:
                        sl = slice(lo, lo + m)
                        base = pos * CHUNK
                        g[base: base + m] = csrc[sl] - (HALF if half else 0)
                        tri_ci[lo:lo + m] = ci
                        tri_p[lo:lo + m] = np.arange(m)
                        if want_sel:
                            perm[ci, :m] = cord[sl]
                    ci += 1
        assert ci == C_total
        crd_l = crd.astype(np.int64)
        d = {
            "gl": _wrap16(gl),
            "gh": _wrap16(gh),
            "perm": perm,
        }
        if want_sel:
            ind = np.zeros((P, C_total * P), F8)
            ind[tri_p, tri_ci * P + crd_l] = 1
            ind_t = np.zeros((P, C_total * P), F8)
            ind_t[crd_l, tri_ci * P + tri_p] = 1
            d["ind"] = ind
            d["ind_t"] = ind_t
        else:
            rda = np.full((P, C_total), -1.0, np.float32)
            rda[tri_p, tri_ci] = crd_l
            d["rd"] = rda
        cores.append(d)
    meta = {
        "KL": KL, "KH": KH, "C": C_total,
        "ncall_low": ncall_low, "ncall_high": ncall_high,
    }
    return cores, meta


# ---------------------------------------------------------------- K1: quantize
def _build_k1():
    nc = bacc.Bacc("TRN2", target_bir_lowering=False, debug=False)
    ins = {}
    outs = {}
    mds = {}
    for g in ("X", "Y"):
        ins[g] = nc.declare_dram_parameter(f"in{g}", [P, R], f32, isOutput=False)
        mds[g] = nc.declare_dram_parameter(f"md{g}", [P, 2 * NB + 1], f32,
                                           isOutput=False)
        outs[g] = nc.declare_dram_parameter(f"out{g}", [R, P], bf16, isOutput=True)

    with tile.TileContext(nc) as tc:
        with (
            tc.tile_pool(name="sb", bufs=1) as sb,
            tc.tile_pool(name="wk", bufs=2) as wk,
            tc.tile_pool(name="acc", bufs=2) as accp,
            tc.tile_pool(name="ob", bufs=1) as obp,
            tc.tile_pool(name="ps", bufs=4, space="PSUM") as ps,
        ):
            idt = sb.tile([P, P], f32)
            make_identity(nc, idt[:])
            idt16 = sb.tile([P, P], f16)
            nc.vector.tensor_copy(out=idt16[:], in_=idt[:])

            for g in ("X", "Y"):
                md = sb.tile([P, 2 * NB + 1], f32, tag=f"md{g}")
                nc.sync.dma_start(out=md[:], in_=mds[g][:, :])
                raw = wk.tile([P, R], f32, tag="raw")
                nc.sync.dma_start(out=raw[:], in_=ins[g][:, :])
                tf = wk.tile([P, R], f16, tag="tf")
                nc.scalar.activation(out=tf[:], in_=raw[:],
                                     func=mybir.ActivationFunctionType.Sigmoid)
                acc = accp.tile([P, R], f16, tag="acc")
                # acc = 0*tf + base
                nc.vector.tensor_scalar(out=acc[:], in0=tf[:], scalar1=0.0,
                                        scalar2=md[:, 2 * NB:2 * NB + 1],
                                        op0=mybir.AluOpType.mult,
                                        op1=mybir.AluOpType.add)
                for j in range(NB):
                    t1 = wk.tile([P, R], f16, tag="t1")
                    nc.vector.tensor_scalar(out=t1[:], in0=tf[:],
                                            scalar1=md[:, j:j + 1],
                                            scalar2=md[:, NB + j:NB + j + 1],
                                            op0=mybir.AluOpType.is_gt,
                                            op1=mybir.AluOpType.mult)
                    acc2 = accp.tile([P, R], f16, tag="acc")
                    nc.vector.tensor_tensor(out=acc2[:], in0=acc[:], in1=t1[:],
                                            op=mybir.AluOpType.add)
                    acc = acc2
                ob = obp.tile([P, W, P], bf16, tag="ob")
                for w in range(W):
                    pt = ps.tile([P, P], f16, tag="pt")
                    nc.tensor.transpose(out=pt[:],
                                        in_=acc[:, w * P:(w + 1) * P],
                                        identity=idt16[:])
                    nc.vector.tensor_copy(out=ob[:, w, :], in_=pt[:])
                nc.sync.dma_start(
                    out=outs[g].rearrange("(w p) c -> p w c", p=P),
                    in_=ob[:])
    nc.compile()
    return nc


# ---------------------------------------------------------------- KB: scatter Y
def _build_kb(meta):
    nc = bacc.Bacc("TRN2", target_bir_lowering=False, debug=False,
                   num_swdge_queues=4)
    KL, KH, C = meta["KL"], meta["KH"], meta["C"]
    ncl, nch = meta["ncall_low"], meta["ncall_high"]
    tab = nc.declare_dram_parameter("tab", [NRY, P], bf16, isOutput=False)
    py = nc.declare_dram_parameter("py", [P, W, F], bf16, isOutput=False)
    rsq = nc.declare_dram_parameter("rsq", [P, W], f32, isOutput=False)
    gl = nc.declare_dram_parameter("gl", [P, ncl * 64], i16, isOutput=False)
    gh = nc.declare_dram_parameter("gh", [P, nch * 64], i16, isOutput=False)
    rdd = nc.declare_dram_parameter("rd", [P, C], f32, isOutput=False)
    yout = nc.declare_dram_parameter("yout", [R, F], bf16, isOutput=True)

    with tile.TileContext(nc) as tc:
        with (
            tc.tile_pool(name="sb", bufs=1) as sb,
            tc.tile_pool(name="gt", bufs=16) as gt,
            tc.tile_pool(name="indp", bufs=4) as indp,
            tc.tile_pool(name="fl", bufs=2) as fl,
            tc.tile_pool(name="ps", bufs=2, space="PSUM") as ps,
        ):
            gl_t = sb.tile([P, ncl * 64], i16)
            nc.sync.dma_start(out=gl_t[:], in_=gl[:, :])
            gh_t = sb.tile([P, nch * 64], i16)
            nc.sync.dma_start(out=gh_t[:], in_=gh[:, :])
            py_t = sb.tile([P, W, F], bf16)
            nc.sync.dma_start(out=py_t[:], in_=py[:, :, :])
            rsq_t = sb.tile([P, W], f32)
            nc.sync.dma_start(out=rsq_t[:], in_=rsq[:, :])
            rd_t = sb.tile([P, C], f32)
            nc.sync.dma_start(out=rd_t[:], in_=rdd[:, :])
            ob = sb.tile([P, W, F], bf16)
            iota_i = sb.tile([P, P], mybir.dt.int32)
            nc.gpsimd.iota(iota_i[:], pattern=[[1, P]], base=0,
                           channel_multiplier=0)
            iotaF = sb.tile([P, P], bf16)
            nc.vector.tensor_copy(out=iotaF[:], in_=iota_i[:])

            srcs = {0: (tab[0:HALF, :], gl_t), 1: (tab[HALF:NRY, :], gh_t)}
            call_tiles = {}
            qctr = [0]

            def get_blk(stream, pos):
                call = pos // CALL
                key = (stream, call)
                if key not in call_tiles:
                    src_ap, idx_t = srcs[stream]
                    g = gt.tile([P, CALL, P], bf16, tag="g")
                    nc.gpsimd.dma_gather(
                        out_ap=g[:], in_ap=src_ap,
                        idxs_ap=idx_t[:, call * 64:(call + 1) * 64],
                        num_idxs=CALL * CHUNK, num_idxs_reg=CALL * CHUNK,
                        elem_size=P, queue_num=qctr[0] % 4)
                    qctr[0] += 1
                    call_tiles[key] = g
                return call_tiles[key], pos % CALL

            ci = 0
            pl = ph = 0
            for wi in range(W):
                Kw = int(KL[wi] + KH[wi])
                pm = ps.tile([P, F], f32, tag="pm")
                for c in range(Kw):
                    if c < KL[wi]:
                        gtile, blk = get_blk(0, pl)
                        pl += 1
                    else:
                        gtile, blk = get_blk(1, ph)
                        ph += 1
                    ind_c = indp.tile([P, P], bf16, tag="ind")
                    nc.vector.tensor_scalar(out=ind_c[:], in0=iotaF[:],
                                            scalar1=rd_t[:, ci:ci + 1],
                                            scalar2=None,
                                            op0=mybir.AluOpType.is_equal)
                    nc.tensor.matmul(pm[:], lhsT=ind_c[:],
                                     rhs=gtile[:, blk, F:P],
                                     start=(c == 0), stop=(c == Kw - 1))
                    ci += 1
                nc.vector.scalar_tensor_tensor(
                    out=ob[:, wi, :], in0=pm[:], scalar=rsq_t[:, wi:wi + 1],
                    in1=py_t[:, wi, :], op0=mybir.AluOpType.mult,
                    op1=mybir.AluOpType.add)
            nc.sync.dma_start(out=yout.rearrange("(w p) c -> p w c", p=P),
                              in_=ob[:])
    nc.compile()
    return nc


# ---------------------------------------------------------------- KC: X + dot
def _build_kc(meta):
    nc = bacc.Bacc("TRN2", target_bir_lowering=False, debug=False,
                   num_swdge_queues=4)
    KL, KH, C = meta["KL"], meta["KH"], meta["C"]
    ncl, nch = meta["ncall_low"], meta["ncall_high"]
    tab = nc.declare_dram_parameter("tab", [NRY, P], bf16, isOutput=False)
    px = nc.declare_dram_parameter("px", [P, W, F], bf16, isOutput=False)
    rsq = nc.declare_dram_parameter("rsq", [P, W], f32, isOutput=False)
    gl = nc.declare_dram_parameter("gl", [P, ncl * 64], i16, isOutput=False)
    gh = nc.declare_dram_parameter("gh", [P, nch * 64], i16, isOutput=False)
    indd = nc.declare_dram_parameter("ind", [P, C * P], fp8, isOutput=False)
    indtd = nc.declare_dram_parameter("ind_t", [P, C * P], fp8, isOutput=False)
    abc = nc.declare_dram_parameter("abc", [P, 3 * F + 1], f32, isOutput=False)
    outv = nc.declare_dram_parameter("outv", [P, C], f32, isOutput=True)
    rout = nc.declare_dram_parameter("rout", [P, W], f32, isOutput=True)

    with tile.TileContext(nc) as tc:
        with (
            tc.tile_pool(name="sb", bufs=1) as sb,
            tc.tile_pool(name="gt", bufs=30) as gt,
            tc.tile_pool(name="indp", bufs=3) as indp,
            tc.tile_pool(name="intp", bufs=3) as intp,
            tc.tile_pool(name="fl", bufs=3) as fl,
            tc.tile_pool(name="jk", bufs=4) as jkp,
            tc.tile_pool(name="ps", bufs=2, space="PSUM") as ps,
            tc.tile_pool(name="ps2", bufs=4, space="PSUM") as ps2,
        ):
            gl_t = sb.tile([P, ncl * 64], i16)
            nc.sync.dma_start(out=gl_t[:], in_=gl[:, :])
            gh_t = sb.tile([P, nch * 64], i16)
            nc.sync.dma_start(out=gh_t[:], in_=gh[:, :])
            px_t = sb.tile([P, W, F], bf16)
            nc.sync.dma_start(out=px_t[:], in_=px[:, :, :])
            rsq_t = sb.tile([P, W], f32)
            nc.sync.dma_start(out=rsq_t[:], in_=rsq[:, :])
            abc_t = sb.tile([P, 3 * F + 1], f32)
            nc.sync.dma_start(out=abc_t[:], in_=abc[:, :])
            At, Bt, Ct = (abc_t[:, 0:F], abc_t[:, F:2 * F], abc_t[:, 2 * F:3 * F])
            ot = sb.tile([P, C], f32)
            rb = sb.tile([P, W], f32)

            srcs = {0: (tab[0:HALF, :], gl_t), 1: (tab[HALF:NRY, :], gh_t)}
            call_tiles = {}
            qctr = [0]

            def get_blk(stream, pos):
                call = pos // CALL
                key = (stream, call)
                if key not in call_tiles:
                    src_ap, idx_t = srcs[stream]
                    g = gt.tile([P, CALL, P], bf16, tag="g")
                    nc.gpsimd.dma_gather(
                        out_ap=g[:], in_ap=src_ap,
                        idxs_ap=idx_t[:, call * 64:(call + 1) * 64],
                        num_idxs=CALL * CHUNK, num_idxs_reg=CALL * CHUNK,
                        elem_size=P, queue_num=qctr[0] % 4)
                    qctr[0] += 1
                    call_tiles[key] = g
                return call_tiles[key], pos % CALL

            Kmax = int((KL + KH).max())
            ci = 0
            pl = ph = 0
            for wi in range(W):
                Kw = int(KL[wi] + KH[wi])
                ind_w = indp.tile([P, Kmax, P], fp8, tag="ind")
                nc.sync.dma_start(out=ind_w[:, 0:Kw, :],
                                  in_=indd[:, ci * P:(ci + Kw) * P])
                indt_w = intp.tile([P, Kmax, P], fp8, tag="indt")
                nc.scalar.dma_start(out=indt_w[:, 0:Kw, :],
                                    in_=indtd[:, ci * P:(ci + Kw) * P])
                pm = ps.tile([P, F], f32, tag="pm")
                blks = []
                for c in range(Kw):
                    if c < KL[wi]:
                        gtile, blk = get_blk(0, pl)
                        pl += 1
                    else:
                        gtile, blk = get_blk(1, ph)
                        ph += 1
                    blks.append((gtile, blk))
                    nc.tensor.matmul(pm[:], lhsT=ind_w[:, c, :],
                                     rhs=gtile[:, blk, 0:F],
                                     start=(c == 0), stop=(c == Kw - 1))
                # flush window: X, XA65
                Xw = fl.tile([P, F], f32, tag="X")
                nc.vector.scalar_tensor_tensor(
                    out=Xw[:], in0=pm[:], scalar=rsq_t[:, wi:wi + 1],
                    in1=px_t[:, wi, :], op0=mybir.AluOpType.mult,
                    op1=mybir.AluOpType.add)
                xa = fl.tile([P, F], bf16, tag="xa")
                xt = fl.tile([P, F], f32, tag="xt")
                nc.vector.tensor_tensor(out=xt[:], in0=Xw[:], in1=At,
                                        op=mybir.AluOpType.mult)
                nc.vector.tensor_tensor(out=xa[:], in0=xt[:], in1=Ct,
                                        op=mybir.AluOpType.add)
                jk0 = jkp.tile([P, F], f32, tag="jk0")
                nc.vector.scalar_tensor_tensor(
                    out=jk0[:], in0=Xw[:], scalar=1.0, in1=Bt,
                    op0=mybir.AluOpType.mult, op1=mybir.AluOpType.mult,
                    accum_out=rb[:, wi:wi + 1])
                # select + dot per chunk
                for c in range(Kw):
                    gtile, blk = blks[c]
                    sel = ps2.tile([P, F], f32, tag="sel")
                    nc.tensor.matmul(sel[:], lhsT=indt_w[:, c, :],
                                     rhs=xa[:], start=True, stop=True)
                    jk = jkp.tile([P, F], f32, tag="jk")
                    nc.vector.scalar_tensor_tensor(
                        out=jk[:], in0=sel[:], scalar=1.0,
                        in1=gtile[:, blk, F:P],
                        op0=mybir.AluOpType.mult, op1=mybir.AluOpType.mult,
                        accum_out=ot[:, ci + c:ci + c + 1])
                ci += Kw
            nc.sync.dma_start(out=outv[:, :], in_=ot[:])
            nc.sync.dma_start(out=rout[:, :], in_=rb[:])
    nc.compile()
    return nc


# ---------------------------------------------------------------- entry point
def kernel(feats, ifeats, keys, ikeys, values, scale, idxs):
    feats = np.asarray(feats, np.float32)
    ifeats = np.asarray(ifeats, np.float32)
    keys = np.asarray(keys, np.float32)
    ikeys = np.asarray(ikeys, np.float32)
    values = np.asarray(values, np.float32)
    scale = np.asarray(scale, np.float32)
    i0 = np.asarray(idxs[0]).astype(np.int64).astype(np.int32)
    i1 = np.asarray(idxs[1]).astype(np.int64).astype(np.int32)
    N = len(i0)
    LAST_HW_NS.clear()

    # ---- host: codebook boundary metadata (md = [m(15) | d(15) | base])
    def md_cols(k_raw, axis, off):
        tk = 1.0 / (1.0 + np.exp(-np.sort(k_raw[axis], axis=-1)))  # (F, NQ)
        m = 0.5 * (tk[:, :NB] + tk[:, 1:])
        d = tk[:, 1:] - tk[:, :NB]
        base = tk[:, 0:1] + off
        return np.concatenate([m, d, base], axis=1)  # (F, 31)

    mdX = np.concatenate([md_cols(keys, 0, 0.0), md_cols(ikeys, 0, -0.5)],
                         axis=0).astype(np.float32)      # (128, 31) px|ipx
    mdY = np.concatenate([md_cols(keys, 1, 0.0), md_cols(ikeys, 1, -0.5)],
                         axis=0).astype(np.float32)      # (128, 31) py|ipy

    # ---- host: K1 input shards, features on partitions (top=feats, bot=ifeats)
    def shard_T(base, k):
        lo = base + k * R
        hi = min(base + D0, lo + R)
        out = np.zeros((P, R), np.float32)
        n = hi - lo
        if n > 0:
            out[0:F, 0:n] = feats[lo:hi].T
            out[F:P, 0:n] = ifeats[lo:hi].T
        return out

    nc1 = _build_k1()
    maps1 = []
    for k in range(NC):
        maps1.append({
            "inX": shard_T(0, k), "mdX": mdX,
            "inY": shard_T(D0, k), "mdY": mdY,
        })
    r1 = _run(nc1, maps1)

    # outX rows: [px | ipx]; outY rows: [py | ipy]  (R, 128) per core
    tabB = np.concatenate([r1[k]["outX"] for k in range(NC)], axis=0)
    outY = [r1[k]["outY"] for k in range(NC)]

    # ---- host: edge packing + count metadata
    cnt0 = np.bincount(i0, minlength=NRY).astype(np.float64)
    cnt1 = np.bincount(i1, minlength=NRY).astype(np.float64)
    rsq0 = (1.0 / np.sqrt(cnt0 + 1e-12)).astype(np.float32)
    rsq1 = (1.0 / np.sqrt(cnt1 + 1e-12)).astype(np.float32)

    def rsq_shard(rsq_full, k):
        return rsq_full[k * R:(k + 1) * R].reshape(W, P).T.copy()  # (P, W)

    coresB, metaB = _pack_pass(i1, i0, want_sel=False)  # dest=i1, gather i0
    coresC, metaC = _pack_pass(i0, i1, want_sel=True)   # dest=i0, gather i1

    # ---- KB
    ncb = _build_kb(metaB)
    maps2 = []
    for k in range(NC):
        py_k = outY[k][:, 0:F].reshape(W, P, F).transpose(1, 0, 2).copy()
        maps2.append({
            "tab": tabB,
            "py": py_k,
            "rsq": rsq_shard(rsq1, k),
            "gl": coresB[k]["gl"], "gh": coresB[k]["gh"],
            "rd": coresB[k]["rd"],
        })
    r2 = _run(ncb, maps2)

    # tabC rows: [ipy | Y]
    y_full = np.concatenate([r2[k]["yout"] for k in range(NC)], axis=0)
    ipy_full = np.concatenate([outY[k][:, F:P] for k in range(NC)], axis=0)
    tabC = np.concatenate([ipy_full, y_full], axis=1)  # (NRY, 128)

    # ---- host: folded values/scale constants
    s = float(scale[0])
    V = values[0]  # (4, F)
    Arow = s * (V[0] - V[1] - V[2] + V[3])
    Brow = s * (V[1] - V[3])
    Crow = s * (V[2] - V[3])
    sumD = s * float(V[3].sum())
    abc = np.concatenate([
        np.tile(Arow, (P, 1)), np.tile(Brow, (P, 1)), np.tile(Crow, (P, 1)),
        np.full((P, 1), sumD, np.float32)], axis=1).astype(np.float32)

    # ---- KC
    ncc = _build_kc(metaC)
    maps3 = []
    for k in range(NC):
        px_k = r1[k]["outX"][:, 0:F].reshape(W, P, F).transpose(1, 0, 2).copy()
        maps3.append({
            "tab": tabC,
            "px": px_k,
            "rsq": rsq_shard(rsq0, k),
            "gl": coresC[k]["gl"], "gh": coresC[k]["gh"],
            "ind": coresC[k]["ind"], "ind_t": coresC[k]["ind_t"],
            "abc": abc,
        })
    r3 = _run(ncc, maps3)

    # r_full[d] = B·X[d] (device) + sumD (host)
    r_full = np.concatenate(
        [r3[k]["rout"].T.reshape(R) for k in range(NC)]) + np.float32(sumD)

    out = np.zeros(N, np.float32)
    for k in range(NC):
        vals = r3[k]["outv"]          # (P, C)
        perm = coresC[k]["perm"]      # (C, P)
        m = perm >= 0
        out[perm[m]] = vals.T[m]
    out += r_full[i0]
    return out


# revision 21
# speedup vs baseline: 1.3393x; 1.3393x over previous
"""Trainium2 Bass kernel for nn_ELiCiT_50087908606687 (vq_codebook), v2.

Math (forward only): with X = p0 + S0·rsq0 (mode-0 node table),
Y = p1 + S1·rsq1 (mode-1), S0[d] = sum_{e:i0=d} (ipy[i1[e]]-0.5), the
reference einsum collapses per edge n to:

    out[n] = sum_f (A_f X[i0,f] + C_f) Y[i1,f] + (B·X[i0] + sumD)

with A = s(V0-V1-V2+V3), B = s(V1-V3), C = s(V2-V3), sumD = s·sum(V3)
(scale s folded in). The -0.5 offsets are baked into the stored ip
tables; rsq/count metadata is host-derived from idxs alone.

Device decomposition (8 cores, 3 SPMD launches, dest-range sharding):
  K1: quantize, features-on-partition: per boundary one fused
      tensor_scalar (is_gt ×delta, 4x DVE mode) + one f16 add; PE
      transposes back to row-major [px|ipx], [py|ipy] tables.
  KB: per core (i1 ranges): dma_gather [px|ipx] rows at i0 (one 256B/edge
      stream), one-hot matmuls accumulate S1 in PSUM per 128-row window,
      flush Y = py + rsq1·S1.
  KC: per core (i0 ranges): gather [ipy|Y] rows at i1; scatter matmuls
      (host-built one-hot ind) → S0 → X → XA65=[A∘X+C | B·X+sumD];
      selection matmuls (host-built ind_t) pick XA65 rows per edge;
      one tensor_tensor_reduce per chunk computes the 64-dot + r
      against the kept gather tiles' Y half.
Host does sharding, sorting/packing of index streams, one-hot masks,
bincounts, and inter-launch assembly only.
"""
import sys

sys.path.insert(0, "/opt/trn_rl_repo")

import os

import numpy as np
import ml_dtypes

try:
    import prof_shim  # noqa: F401  (registers NTFF hook when available)
except Exception:
    pass

import concourse.bacc as bacc
import concourse.mybir as mybir
import concourse.tile as tile
from concourse.bass_utils import run_bass_kernel_spmd
from concourse.masks import make_identity

TRACE = bool(int(os.environ.get("KERNEL_TRACE", "0")))
LAST_HW_NS = []

NC = 8
D0 = 50000
P = 128
F = 64
NQ = 16
NB = 15
R = 6272          # dest rows per core (49 windows of 128)
W = 49
HALF = 32768      # int16 split point for full-range gathers
NRY = NC * R      # padded full-table rows (50176)
CHUNK = 128
CALL = 8          # chunks per dma_gather call (1024 idxs)

f32 = mybir.dt.float32
bf16 = mybir.dt.bfloat16
f16 = mybir.dt.float16
i16 = mybir.dt.int16
BF = ml_dtypes.bfloat16
F8 = ml_dtypes.float8_e4m3
fp8 = mybir.dt.float8e4


def _run(nc, maps):
    r = run_bass_kernel_spmd(nc, maps, list(range(NC)), trace=TRACE)
    if TRACE:
        LAST_HW_NS.append(r.exec_time_ns or 0)
    return r.results


# ---------------------------------------------------------------- host utils
def _wrap16(flat):
    """Pack flat idx list (multiple of 1024) into (128, n*64) int16 SWDGE
    layout: per 1024-call, j -> [j % 16, j // 16], replicated 8x down."""
    ncall = len(flat) // 1024
    cols = []
    for c in range(ncall):
        a = flat[c * 1024:(c + 1) * 1024].reshape(64, 16).T  # (16, 64)
        cols.append(np.tile(a, (8, 1)))                      # (128, 64)
    return np.concatenate(cols, axis=1).astype(np.int16)     # (128, ncall*64)


def _pack_pass(dst, src, want_sel):
    """Bucket edges by dest core/window, order low/high by src-half, pad.

    Returns per-core dicts: gl/gh (wrap16 idx streams), ind (P, C*128)
    one-hot [edge-partition, destrow-free] bf16, ind_t transposed, perm,
    plus meta with per-window chunk counts and window→chunk spans.
    """
    core = np.minimum(dst // R, NC - 1)
    loc = dst - core * R
    w = loc // CHUNK
    rd = loc % CHUNK
    hi = (src >= HALF).astype(np.int8)

    order = np.lexsort((hi, w, core))
    oc, ow, ohi = core[order], w[order], hi[order]
    ord_src, ord_rd = src[order], rd[order]

    cnt = np.zeros((NC, W, 2), np.int64)
    np.add.at(cnt, (oc, ow, ohi), 1)
    KL = np.maximum(np.ceil(cnt[:, :, 0] / CHUNK).astype(np.int64).max(axis=0), 1)
    KH = np.maximum(np.ceil(cnt[:, :, 1] / CHUNK).astype(np.int64).max(axis=0), 1)

    C_total = int((KL + KH).sum())
    nlow = int(KL.sum())
    nhigh = int(KH.sum())
    ncall_low = -(-nlow // CALL)
    ncall_high = -(-nhigh // CALL)

    # stream position of each (window, chunk-within-half)
    low_pos = {}
    high_pos = {}
    pl = ph = 0
    for wi in range(W):
        for c in range(int(KL[wi])):
            low_pos[(wi, c)] = pl
            pl += 1
        for c in range(int(KH[wi])):
            high_pos[(wi, c)] = ph
            ph += 1

    cores = []
    core_starts = np.searchsorted(oc, np.arange(NC + 1))
    for k in range(NC):
        s, e = core_starts[k], core_starts[k + 1]
        cw, chi = ow[s:e], ohi[s:e]
        csrc, crd, cord = ord_src[s:e], ord_rd[s:e], order[s:e]

        gl = np.zeros(ncall_low * 1024, np.int64)
        gh = np.zeros(ncall_high * 1024, np.int64)
        # flat (ci, p, rdval) triples for one-hot builds
        tri_ci = np.empty(e - s, np.int64)
        tri_p = np.empty(e - s, np.int64)
        perm = np.full((C_total, P), -1, np.int64) if want_sel else None

        key = cw * 2 + chi
        starts = np.searchsorted(key, np.arange(2 * W + 1))
        ci = 0
        for wi in range(W):
            for half, K_half, posmap, g in ((0, int(KL[wi]), low_pos, gl),
                                            (1, int(KH[wi]), high_pos, gh)):
                a, b = starts[wi * 2 + half], starts[wi * 2 + half + 1]
                n = b - a
                for c in range(K_half):
                    lo = a + c * CHUNK
                    m = min(CHUNK, max(0, n - c * CHUNK))
                    pos = posmap[(wi, c)]
                    if m > 0:
                        sl = slice(lo, lo + m)
                        base = pos * CHUNK
                        g[base: base + m] = csrc[sl] - (HALF if half else 0)
                        tri_ci[lo:lo + m] = ci
                        tri_p[lo:lo + m] = np.arange(m)
                        if want_sel:
                            perm[ci, :m] = cord[sl]
                    ci += 1
        assert ci == C_total
        crd_l = crd.astype(np.int64)
        d = {
            "gl": _wrap16(gl),
            "gh": _wrap16(gh),
            "perm": perm,
        }
        ind = np.zeros((P, C_total * P), F8)
        ind[tri_p, tri_ci * P + crd_l] = 1
        d["ind"] = ind
        if want_sel:
            ind_t = np.zeros((P, C_total * P), F8)
            ind_t[crd_l, tri_ci * P + tri_p] = 1
            d["ind_t"] = ind_t
        cores.append(d)
    meta = {
        "KL": KL, "KH": KH, "C": C_total,
        "ncall_low": ncall_low, "ncall_high": ncall_high,
    }
    return cores, meta


# ---------------------------------------------------------------- K1: quantize
def _build_k1():
    nc = bacc.Bacc("TRN2", target_bir_lowering=False, debug=False)
    ins = {}
    outs = {}
    mds = {}
    for g in ("X", "Y"):
        ins[g] = nc.declare_dram_parameter(f"in{g}", [P, R], f32, isOutput=False)
        mds[g] = nc.declare_dram_parameter(f"md{g}", [P, 2 * NB + 1], f32,
                                           isOutput=False)
        outs[g] = nc.declare_dram_parameter(f"out{g}", [R, P], bf16, isOutput=True)

    with tile.TileContext(nc) as tc:
        with (
            tc.tile_pool(name="sb", bufs=1) as sb,
            tc.tile_pool(name="wk", bufs=2) as wk,
            tc.tile_pool(name="acc", bufs=2) as accp,
            tc.tile_pool(name="ob", bufs=1) as obp,
            tc.tile_pool(name="ps", bufs=4, space="PSUM") as ps,
        ):
            idt = sb.tile([P, P], f32)
            make_identity(nc, idt[:])
            idt16 = sb.tile([P, P], f16)
            nc.vector.tensor_copy(out=idt16[:], in_=idt[:])

            for g in ("X", "Y"):
                md = sb.tile([P, 2 * NB + 1], f32, tag=f"md{g}")
                nc.sync.dma_start(out=md[:], in_=mds[g][:, :])
                raw = wk.tile([P, R], f32, tag="raw")
                nc.sync.dma_start(out=raw[:], in_=ins[g][:, :])
                tf = wk.tile([P, R], f16, tag="tf")
                nc.scalar.activation(out=tf[:], in_=raw[:],
                                     func=mybir.ActivationFunctionType.Sigmoid)
                acc = accp.tile([P, R], f16, tag="acc")
                # acc = 0*tf + base
                nc.vector.tensor_scalar(out=acc[:], in0=tf[:], scalar1=0.0,
                                        scalar2=md[:, 2 * NB:2 * NB + 1],
                                        op0=mybir.AluOpType.mult,
                                        op1=mybir.AluOpType.add)
                for j in range(NB):
                    t1 = wk.tile([P, R], f16, tag="t1")
                    nc.vector.tensor_scalar(out=t1[:], in0=tf[:],
                                            scalar1=md[:, j:j + 1],
                                            scalar2=md[:, NB + j:NB + j + 1],
                                            op0=mybir.AluOpType.is_gt,
                                            op1=mybir.AluOpType.mult)
                    acc2 = accp.tile([P, R], f16, tag="acc")
                    nc.vector.tensor_tensor(out=acc2[:], in0=acc[:], in1=t1[:],
                                            op=mybir.AluOpType.add)
                    acc = acc2
                ob = obp.tile([P, W, P], bf16, tag="ob")
                for w in range(W):
                    pt = ps.tile([P, P], f16, tag="pt")
                    nc.tensor.transpose(out=pt[:],
                                        in_=acc[:, w * P:(w + 1) * P],
                                        identity=idt16[:])
                    nc.vector.tensor_copy(out=ob[:, w, :], in_=pt[:])
                nc.sync.dma_start(
                    out=outs[g].rearrange("(w p) c -> p w c", p=P),
                    in_=ob[:])
    nc.compile()
    return nc


# ---------------------------------------------------------------- KB: scatter Y
def _build_kb(meta):
    nc = bacc.Bacc("TRN2", target_bir_lowering=False, debug=False,
                   num_swdge_queues=4)
    KL, KH, C = meta["KL"], meta["KH"], meta["C"]
    ncl, nch = meta["ncall_low"], meta["ncall_high"]
    tab = nc.declare_dram_parameter("tab", [NRY, P], bf16, isOutput=False)
    py = nc.declare_dram_parameter("py", [P, W, F], bf16, isOutput=False)
    rsq = nc.declare_dram_parameter("rsq", [P, W], f32, isOutput=False)
    gl = nc.declare_dram_parameter("gl", [P, ncl * 64], i16, isOutput=False)
    gh = nc.declare_dram_parameter("gh", [P, nch * 64], i16, isOutput=False)
    indd = nc.declare_dram_parameter("ind", [P, C * P], fp8, isOutput=False)
    yout = nc.declare_dram_parameter("yout", [R, F], bf16, isOutput=True)

    with tile.TileContext(nc) as tc:
        with (
            tc.tile_pool(name="sb", bufs=1) as sb,
            tc.tile_pool(name="gt", bufs=16) as gt,
            tc.tile_pool(name="indp", bufs=3) as indp,
            tc.tile_pool(name="fl", bufs=2) as fl,
            tc.tile_pool(name="ps", bufs=2, space="PSUM") as ps,
        ):
            gl_t = sb.tile([P, ncl * 64], i16)
            nc.sync.dma_start(out=gl_t[:], in_=gl[:, :])
            gh_t = sb.tile([P, nch * 64], i16)
            nc.sync.dma_start(out=gh_t[:], in_=gh[:, :])
            py_t = sb.tile([P, W, F], bf16)
            nc.sync.dma_start(out=py_t[:], in_=py[:, :, :])
            rsq_t = sb.tile([P, W], f32)
            nc.sync.dma_start(out=rsq_t[:], in_=rsq[:, :])
            ob = sb.tile([P, W, F], bf16)

            srcs = {0: (tab[0:HALF, :], gl_t), 1: (tab[HALF:NRY, :], gh_t)}
            call_tiles = {}
            qctr = [0]

            def get_blk(stream, pos):
                call = pos // CALL
                key = (stream, call)
                if key not in call_tiles:
                    src_ap, idx_t = srcs[stream]
                    g = gt.tile([P, CALL, P], bf16, tag="g")
                    nc.gpsimd.dma_gather(
                        out_ap=g[:], in_ap=src_ap,
                        idxs_ap=idx_t[:, call * 64:(call + 1) * 64],
                        num_idxs=CALL * CHUNK, num_idxs_reg=CALL * CHUNK,
                        elem_size=P, queue_num=qctr[0] % 4)
                    qctr[0] += 1
                    call_tiles[key] = g
                return call_tiles[key], pos % CALL

            Kmax = int((KL + KH).max())
            ci = 0
            pl = ph = 0
            for wi in range(W):
                Kw = int(KL[wi] + KH[wi])
                ind_w = indp.tile([P, Kmax, P], fp8, tag="ind")
                nc.sync.dma_start(out=ind_w[:, 0:Kw, :],
                                  in_=indd[:, ci * P:(ci + Kw) * P])
                pm = ps.tile([P, F], f32, tag="pm")
                for c in range(Kw):
                    if c < KL[wi]:
                        gtile, blk = get_blk(0, pl)
                        pl += 1
                    else:
                        gtile, blk = get_blk(1, ph)
                        ph += 1
                    nc.tensor.matmul(pm[:], lhsT=ind_w[:, c, :],
                                     rhs=gtile[:, blk, F:P],
                                     start=(c == 0), stop=(c == Kw - 1))
                    ci += 1
                nc.vector.scalar_tensor_tensor(
                    out=ob[:, wi, :], in0=pm[:], scalar=rsq_t[:, wi:wi + 1],
                    in1=py_t[:, wi, :], op0=mybir.AluOpType.mult,
                    op1=mybir.AluOpType.add)
            nc.sync.dma_start(out=yout.rearrange("(w p) c -> p w c", p=P),
                              in_=ob[:])
    nc.compile()
    return nc


# ---------------------------------------------------------------- KC: X + dot
def _build_kc(meta):
    nc = bacc.Bacc("TRN2", target_bir_lowering=False, debug=False,
                   num_swdge_queues=4)
    KL, KH, C = meta["KL"], meta["KH"], meta["C"]
    ncl, nch = meta["ncall_low"], meta["ncall_high"]
    tab = nc.declare_dram_parameter("tab", [NRY, P], bf16, isOutput=False)
    px = nc.declare_dram_parameter("px", [P, W, F], bf16, isOutput=False)
    rsq = nc.declare_dram_parameter("rsq", [P, W], f32, isOutput=False)
    gl = nc.declare_dram_parameter("gl", [P, ncl * 64], i16, isOutput=False)
    gh = nc.declare_dram_parameter("gh", [P, nch * 64], i16, isOutput=False)
    indd = nc.declare_dram_parameter("ind", [P, C * P], fp8, isOutput=False)
    indtd = nc.declare_dram_parameter("ind_t", [P, C * P], fp8, isOutput=False)
    abc = nc.declare_dram_parameter("abc", [P, 3 * F + 1], f32, isOutput=False)
    outv = nc.declare_dram_parameter("outv", [P, C], f32, isOutput=True)
    rout = nc.declare_dram_parameter("rout", [P, W], f32, isOutput=True)

    with tile.TileContext(nc) as tc:
        with (
            tc.tile_pool(name="sb", bufs=1) as sb,
            tc.tile_pool(name="gt", bufs=30) as gt,
            tc.tile_pool(name="indp", bufs=3) as indp,
            tc.tile_pool(name="intp", bufs=3) as intp,
            tc.tile_pool(name="fl", bufs=3) as fl,
            tc.tile_pool(name="jk", bufs=4) as jkp,
            tc.tile_pool(name="ps", bufs=2, space="PSUM") as ps,
            tc.tile_pool(name="ps2", bufs=4, space="PSUM") as ps2,
        ):
            gl_t = sb.tile([P, ncl * 64], i16)
            nc.sync.dma_start(out=gl_t[:], in_=gl[:, :])
            gh_t = sb.tile([P, nch * 64], i16)
            nc.sync.dma_start(out=gh_t[:], in_=gh[:, :])
            px_t = sb.tile([P, W, F], bf16)
            nc.sync.dma_start(out=px_t[:], in_=px[:, :, :])
            rsq_t = sb.tile([P, W], f32)
            nc.sync.dma_start(out=rsq_t[:], in_=rsq[:, :])
            abc_t = sb.tile([P, 3 * F + 1], f32)
            nc.sync.dma_start(out=abc_t[:], in_=abc[:, :])
            At, Bt, Ct = (abc_t[:, 0:F], abc_t[:, F:2 * F], abc_t[:, 2 * F:3 * F])
            ot = sb.tile([P, C], f32)
            rb = sb.tile([P, W], f32)

            srcs = {0: (tab[0:HALF, :], gl_t), 1: (tab[HALF:NRY, :], gh_t)}
            call_tiles = {}
            qctr = [0]

            def get_blk(stream, pos):
                call = pos // CALL
                key = (stream, call)
                if key not in call_tiles:
                    src_ap, idx_t = srcs[stream]
                    g = gt.tile([P, CALL, P], bf16, tag="g")
                    nc.gpsimd.dma_gather(
                        out_ap=g[:], in_ap=src_ap,
                        idxs_ap=idx_t[:, call * 64:(call + 1) * 64],
                        num_idxs=CALL * CHUNK, num_idxs_reg=CALL * CHUNK,
                        elem_size=P, queue_num=qctr[0] % 4)
                    qctr[0] += 1
                    call_tiles[key] = g
                return call_tiles[key], pos % CALL

            Kmax = int((KL + KH).max())
            ci = 0
            pl = ph = 0
            for wi in range(W):
                Kw = int(KL[wi] + KH[wi])
                ind_w = indp.tile([P, Kmax, P], fp8, tag="ind")
                nc.sync.dma_start(out=ind_w[:, 0:Kw, :],
                                  in_=indd[:, ci * P:(ci + Kw) * P])
                indt_w = intp.tile([P, Kmax, P], fp8, tag="indt")
                nc.scalar.dma_start(out=indt_w[:, 0:Kw, :],
                                    in_=indtd[:, ci * P:(ci + Kw) * P])
                pm = ps.tile([P, F], f32, tag="pm")
                blks = []
                for c in range(Kw):
                    if c < KL[wi]:
                        gtile, blk = get_blk(0, pl)
                        pl += 1
                    else:
                        gtile, blk = get_blk(1, ph)
                        ph += 1
                    blks.append((gtile, blk))
                    nc.tensor.matmul(pm[:], lhsT=ind_w[:, c, :],
                                     rhs=gtile[:, blk, 0:F],
                                     start=(c == 0), stop=(c == Kw - 1))
                # flush window: X, XA65
                Xw = fl.tile([P, F], f32, tag="X")
                nc.vector.scalar_tensor_tensor(
                    out=Xw[:], in0=pm[:], scalar=rsq_t[:, wi:wi + 1],
                    in1=px_t[:, wi, :], op0=mybir.AluOpType.mult,
                    op1=mybir.AluOpType.add)
                xa = fl.tile([P, F], bf16, tag="xa")
                xt = fl.tile([P, F], f32, tag="xt")
                nc.vector.tensor_tensor(out=xt[:], in0=Xw[:], in1=At,
                                        op=mybir.AluOpType.mult)
                nc.vector.tensor_tensor(out=xa[:], in0=xt[:], in1=Ct,
                                        op=mybir.AluOpType.add)
                jk0 = jkp.tile([P, F], f32, tag="jk0")
                nc.vector.scalar_tensor_tensor(
                    out=jk0[:], in0=Xw[:], scalar=1.0, in1=Bt,
                    op0=mybir.AluOpType.mult, op1=mybir.AluOpType.mult,
                    accum_out=rb[:, wi:wi + 1])
                # select + dot per chunk
                for c in range(Kw):
                    gtile, blk = blks[c]
                    sel = ps2.tile([P, F], f32, tag="sel")
                    nc.tensor.matmul(sel[:], lhsT=indt_w[:, c, :],
                                     rhs=xa[:], start=True, stop=True)
                    jk = jkp.tile([P, F], f32, tag="jk")
                    nc.vector.scalar_tensor_tensor(
                        out=jk[:], in0=sel[:], scalar=1.0,
                        in1=gtile[:, blk, F:P],
                        op0=mybir.AluOpType.mult, op1=mybir.AluOpType.mult,
                        accum_out=ot[:, ci + c:ci + c + 1])
                ci += Kw
            nc.sync.dma_start(out=outv[:, :], in_=ot[:])
            nc.sync.dma_start(out=rout[:, :], in_=rb[:])
    nc.compile()
    return nc


# ---------------------------------------------------------------- entry point
def kernel(feats, ifeats, keys, ikeys, values, scale, idxs):
    feats = np.asarray(feats, np.float32)
    ifeats = np.asarray(ifeats, np.float32)
    keys = np.asarray(keys, np.float32)
    ikeys = np.asarray(ikeys, np.float32)
    values = np.asarray(values, np.float32)
    scale = np.asarray(scale, np.float32)
    i0 = np.asarray(idxs[0]).astype(np.int64).astype(np.int32)
    i1 = np.asarray(idxs[1]).astype(np.int64).astype(np.int32)
    N = len(i0)
    LAST_HW_NS.clear()

    # ---- host: codebook boundary metadata (md = [m(15) | d(15) | base])
    def md_cols(k_raw, axis, off):
        tk = 1.0 / (1.0 + np.exp(-np.sort(k_raw[axis], axis=-1)))  # (F, NQ)
        m = 0.5 * (tk[:, :NB] + tk[:, 1:])
        d = tk[:, 1:] - tk[:, :NB]
        base = tk[:, 0:1] + off
        return np.concatenate([m, d, base], axis=1)  # (F, 31)

    mdX = np.concatenate([md_cols(keys, 0, 0.0), md_cols(ikeys, 0, -0.5)],
                         axis=0).astype(np.float32)      # (128, 31) px|ipx
    mdY = np.concatenate([md_cols(keys, 1, 0.0), md_cols(ikeys, 1, -0.5)],
                         axis=0).astype(np.float32)      # (128, 31) py|ipy

    # ---- host: K1 input shards, features on partitions (top=feats, bot=ifeats)
    def shard_T(base, k):
        lo = base + k * R
        hi = min(base + D0, lo + R)
        out = np.zeros((P, R), np.float32)
        n = hi - lo
        if n > 0:
            out[0:F, 0:n] = feats[lo:hi].T
            out[F:P, 0:n] = ifeats[lo:hi].T
        return out

    nc1 = _build_k1()
    maps1 = []
    for k in range(NC):
        maps1.append({
            "inX": shard_T(0, k), "mdX": mdX,
            "inY": shard_T(D0, k), "mdY": mdY,
        })
    r1 = _run(nc1, maps1)

    # outX rows: [px | ipx]; outY rows: [py | ipy]  (R, 128) per core
    tabB = np.concatenate([r1[k]["outX"] for k in range(NC)], axis=0)
    outY = [r1[k]["outY"] for k in range(NC)]

    # ---- host: edge packing + count metadata
    cnt0 = np.bincount(i0, minlength=NRY).astype(np.float64)
    cnt1 = np.bincount(i1, minlength=NRY).astype(np.float64)
    rsq0 = (1.0 / np.sqrt(cnt0 + 1e-12)).astype(np.float32)
    rsq1 = (1.0 / np.sqrt(cnt1 + 1e-12)).astype(np.float32)

    def rsq_shard(rsq_full, k):
        return rsq_full[k * R:(k + 1) * R].reshape(W, P).T.copy()  # (P, W)

    coresB, metaB = _pack_pass(i1, i0, want_sel=False)  # dest=i1, gather i0
    coresC, metaC = _pack_pass(i0, i1, want_sel=True)   # dest=i0, gather i1

    # ---- KB
    ncb = _build_kb(metaB)
    maps2 = []
    for k in range(NC):
        py_k = outY[k][:, 0:F].reshape(W, P, F).transpose(1, 0, 2).copy()
        maps2.append({
            "tab": tabB,
            "py": py_k,
            "rsq": rsq_shard(rsq1, k),
            "gl": coresB[k]["gl"], "gh": coresB[k]["gh"],
            "ind": coresB[k]["ind"],
        })
    r2 = _run(ncb, maps2)

    # tabC rows: [ipy | Y]
    y_full = np.concatenate([r2[k]["yout"] for k in range(NC)], axis=0)
    ipy_full = np.concatenate([outY[k][:, F:P] for k in range(NC)], axis=0)
    tabC = np.concatenate([ipy_full, y_full], axis=1)  # (NRY, 128)

    # ---- host: folded values/scale constants
    s = float(scale[0])
    V = values[0]  # (4, F)
    Arow = s * (V[0] - V[1] - V[2] + V[3])
    Brow = s * (V[1] - V[3])
    Crow = s * (V[2] - V[3])
    sumD = s * float(V[3].sum())
    abc = np.concatenate([
        np.tile(Arow, (P, 1)), np.tile(Brow, (P, 1)), np.tile(Crow, (P, 1)),
        np.full((P, 1), sumD, np.float32)], axis=1).astype(np.float32)

    # ---- KC
    ncc = _build_kc(metaC)
    maps3 = []
    for k in range(NC):
        px_k = r1[k]["outX"][:, 0:F].reshape(W, P, F).transpose(1, 0, 2).copy()
        maps3.append({
            "tab": tabC,
            "px": px_k,
            "rsq": rsq_shard(rsq0, k),
            "gl": coresC[k]["gl"], "gh": coresC[k]["gh"],
            "ind": coresC[k]["ind"], "ind_t": coresC[k]["ind_t"],
            "abc": abc,
        })
    r3 = _run(ncc, maps3)

    # r_full[d] = B·X[d] (device) + sumD (host)
    r_full = np.concatenate(
        [r3[k]["rout"].T.reshape(R) for k in range(NC)]) + np.float32(sumD)

    out = np.zeros(N, np.float32)
    for k in range(NC):
        vals = r3[k]["outv"]          # (P, C)
        perm = coresC[k]["perm"]      # (C, P)
        m = perm >= 0
        out[perm[m]] = vals.T[m]
    out += r_full[i0]
    return out


# revision 22
# speedup vs baseline: 1.3483x; 1.0068x over previous
"""Trainium2 Bass kernel for nn_ELiCiT_50087908606687 (vq_codebook), v2.

Math (forward only): with X = p0 + S0·rsq0 (mode-0 node table),
Y = p1 + S1·rsq1 (mode-1), S0[d] = sum_{e:i0=d} (ipy[i1[e]]-0.5), the
reference einsum collapses per edge n to:

    out[n] = sum_f (A_f X[i0,f] + C_f) Y[i1,f] + (B·X[i0] + sumD)

with A = s(V0-V1-V2+V3), B = s(V1-V3), C = s(V2-V3), sumD = s·sum(V3)
(scale s folded in). The -0.5 offsets are baked into the stored ip
tables; rsq/count metadata is host-derived from idxs alone.

Device decomposition (8 cores, 3 SPMD launches, dest-range sharding):
  K1: quantize, features-on-partition: per boundary one fused
      tensor_scalar (is_gt ×delta, 4x DVE mode) + one f16 add; PE
      transposes back to row-major [px|ipx], [py|ipy] tables.
  KB: per core (i1 ranges): dma_gather [px|ipx] rows at i0 (one 256B/edge
      stream), one-hot matmuls accumulate S1 in PSUM per 128-row window,
      flush Y = py + rsq1·S1.
  KC: per core (i0 ranges): gather [ipy|Y] rows at i1; scatter matmuls
      (host-built one-hot ind) → S0 → X → XA65=[A∘X+C | B·X+sumD];
      selection matmuls (host-built ind_t) pick XA65 rows per edge;
      one tensor_tensor_reduce per chunk computes the 64-dot + r
      against the kept gather tiles' Y half.
Host does sharding, sorting/packing of index streams, one-hot masks,
bincounts, and inter-launch assembly only.
"""
import sys

sys.path.insert(0, "/opt/trn_rl_repo")

import os

import numpy as np
import ml_dtypes

try:
    import prof_shim  # noqa: F401  (registers NTFF hook when available)
except Exception:
    pass

import concourse.bacc as bacc
import concourse.mybir as mybir
import concourse.tile as tile
from concourse.bass_utils import run_bass_kernel_spmd
from concourse.masks import make_identity

TRACE = bool(int(os.environ.get("KERNEL_TRACE", "0")))
LAST_HW_NS = []

NC = 8
D0 = 50000
P = 128
F = 64
NQ = 16
NB = 15
R = 6272          # dest rows per core (49 windows of 128)
W = 49
HALF = 32768      # int16 split point for full-range gathers
NRY = NC * R      # padded full-table rows (50176)
CHUNK = 128
CALL = 8          # chunks per dma_gather call (1024 idxs)

f32 = mybir.dt.float32
bf16 = mybir.dt.bfloat16
f16 = mybir.dt.float16
i16 = mybir.dt.int16
BF = ml_dtypes.bfloat16
F8 = ml_dtypes.float8_e4m3
fp8 = mybir.dt.float8e4


def _run(nc, maps):
    r = run_bass_kernel_spmd(nc, maps, list(range(NC)), trace=TRACE)
    if TRACE:
        LAST_HW_NS.append(r.exec_time_ns or 0)
    return r.results


# ---------------------------------------------------------------- host utils
def _wrap16(flat):
    """Pack flat idx list (multiple of 1024) into (128, n*64) int16 SWDGE
    layout: per 1024-call, j -> [j % 16, j // 16], replicated 8x down."""
    ncall = len(flat) // 1024
    cols = []
    for c in range(ncall):
        a = flat[c * 1024:(c + 1) * 1024].reshape(64, 16).T  # (16, 64)
        cols.append(np.tile(a, (8, 1)))                      # (128, 64)
    return np.concatenate(cols, axis=1).astype(np.int16)     # (128, ncall*64)


def _pack_pass(dst, src, want_sel):
    """Bucket edges by dest core/window, order low/high by src-half, pad.

    Returns per-core dicts: gl/gh (wrap16 idx streams), ind (P, C*128)
    one-hot [edge-partition, destrow-free] bf16, ind_t transposed, perm,
    plus meta with per-window chunk counts and window→chunk spans.
    """
    core = np.minimum(dst // R, NC - 1)
    loc = dst - core * R
    w = loc // CHUNK
    rd = loc % CHUNK
    hi = (src >= HALF).astype(np.int8)

    order = np.lexsort((hi, w, core))
    oc, ow, ohi = core[order], w[order], hi[order]
    ord_src, ord_rd = src[order], rd[order]

    cnt = np.zeros((NC, W, 2), np.int64)
    np.add.at(cnt, (oc, ow, ohi), 1)
    KL = np.maximum(np.ceil(cnt[:, :, 0] / CHUNK).astype(np.int64).max(axis=0), 1)
    KH = np.maximum(np.ceil(cnt[:, :, 1] / CHUNK).astype(np.int64).max(axis=0), 1)

    C_total = int((KL + KH).sum())
    nlow = int(KL.sum())
    nhigh = int(KH.sum())
    ncall_low = -(-nlow // CALL)
    ncall_high = -(-nhigh // CALL)

    # stream position of each (window, chunk-within-half)
    low_pos = {}
    high_pos = {}
    pl = ph = 0
    for wi in range(W):
        for c in range(int(KL[wi])):
            low_pos[(wi, c)] = pl
            pl += 1
        for c in range(int(KH[wi])):
            high_pos[(wi, c)] = ph
            ph += 1

    cores = []
    core_starts = np.searchsorted(oc, np.arange(NC + 1))
    for k in range(NC):
        s, e = core_starts[k], core_starts[k + 1]
        cw, chi = ow[s:e], ohi[s:e]
        csrc, crd, cord = ord_src[s:e], ord_rd[s:e], order[s:e]

        gl = np.zeros(ncall_low * 1024, np.int64)
        gh = np.zeros(ncall_high * 1024, np.int64)
        # flat (ci, p, rdval) triples for one-hot builds
        tri_ci = np.empty(e - s, np.int64)
        tri_p = np.empty(e - s, np.int64)
        perm = np.full((C_total, P), -1, np.int64) if want_sel else None

        key = cw * 2 + chi
        starts = np.searchsorted(key, np.arange(2 * W + 1))
        ci = 0
        for wi in range(W):
            for half, K_half, posmap, g in ((0, int(KL[wi]), low_pos, gl),
                                            (1, int(KH[wi]), high_pos, gh)):
                a, b = starts[wi * 2 + half], starts[wi * 2 + half + 1]
                n = b - a
                for c in range(K_half):
                    lo = a + c * CHUNK
                    m = min(CHUNK, max(0, n - c * CHUNK))
                    pos = posmap[(wi, c)]
                    if m > 0:
                        sl = slice(lo, lo + m)
                        base = pos * CHUNK
                        g[base: base + m] = csrc[sl] - (HALF if half else 0)
                        tri_ci[lo:lo + m] = ci
                        tri_p[lo:lo + m] = np.arange(m)
                        if want_sel:
                            perm[ci, :m] = cord[sl]
                    ci += 1
        assert ci == C_total
        crd_l = crd.astype(np.int64)
        d = {
            "gl": _wrap16(gl),
            "gh": _wrap16(gh),
            "perm": perm,
        }
        ind = np.zeros((P, C_total * P), F8)
        ind[tri_p, tri_ci * P + crd_l] = 1
        d["ind"] = ind
        if want_sel:
            ind_t = np.zeros((P, C_total * P), F8)
            ind_t[crd_l, tri_ci * P + tri_p] = 1
            d["ind_t"] = ind_t
        cores.append(d)
    meta = {
        "KL": KL, "KH": KH, "C": C_total,
        "ncall_low": ncall_low, "ncall_high": ncall_high,
    }
    return cores, meta


# ---------------------------------------------------------------- K1: quantize
def _build_k1():
    nc = bacc.Bacc("TRN2", target_bir_lowering=False, debug=False)
    ins = {}
    outs = {}
    mds = {}
    for g in ("X", "Y"):
        ins[g] = nc.declare_dram_parameter(f"in{g}", [P, R], f32, isOutput=False)
        mds[g] = nc.declare_dram_parameter(f"md{g}", [P, 2 * NB + 1], f32,
                                           isOutput=False)
        outs[g] = nc.declare_dram_parameter(f"out{g}", [R, P], bf16, isOutput=True)

    with tile.TileContext(nc) as tc:
        with (
            tc.tile_pool(name="sb", bufs=1) as sb,
            tc.tile_pool(name="wk", bufs=2) as wk,
            tc.tile_pool(name="acc", bufs=2) as accp,
            tc.tile_pool(name="ob", bufs=1) as obp,
            tc.tile_pool(name="ps", bufs=4, space="PSUM") as ps,
        ):
            idt = sb.tile([P, P], f32)
            make_identity(nc, idt[:])
            idt16 = sb.tile([P, P], f16)
            nc.vector.tensor_copy(out=idt16[:], in_=idt[:])

            for g in ("X", "Y"):
                md = sb.tile([P, 2 * NB + 1], f32, tag=f"md{g}")
                nc.sync.dma_start(out=md[:], in_=mds[g][:, :])
                raw = wk.tile([P, R], f32, tag="raw")
                nc.sync.dma_start(out=raw[:], in_=ins[g][:, :])
                tf = wk.tile([P, R], f16, tag="tf")
                nc.scalar.activation(out=tf[:], in_=raw[:],
                                     func=mybir.ActivationFunctionType.Sigmoid)
                acc = accp.tile([P, R], f16, tag="acc")
                # acc = 0*tf + base
                nc.vector.tensor_scalar(out=acc[:], in0=tf[:], scalar1=0.0,
                                        scalar2=md[:, 2 * NB:2 * NB + 1],
                                        op0=mybir.AluOpType.mult,
                                        op1=mybir.AluOpType.add)
                for j in range(NB):
                    t1 = wk.tile([P, R], f16, tag="t1")
                    nc.vector.tensor_scalar(out=t1[:], in0=tf[:],
                                            scalar1=md[:, j:j + 1],
                                            scalar2=md[:, NB + j:NB + j + 1],
                                            op0=mybir.AluOpType.is_gt,
                                            op1=mybir.AluOpType.mult)
                    acc2 = accp.tile([P, R], f16, tag="acc")
                    nc.vector.tensor_tensor(out=acc2[:], in0=acc[:], in1=t1[:],
                                            op=mybir.AluOpType.add)
                    acc = acc2
                ob = obp.tile([P, W, P], bf16, tag="ob")
                for w in range(W):
                    pt = ps.tile([P, P], f16, tag="pt")
                    nc.tensor.transpose(out=pt[:],
                                        in_=acc[:, w * P:(w + 1) * P],
                                        identity=idt16[:])
                    nc.vector.tensor_copy(out=ob[:, w, :], in_=pt[:])
                nc.sync.dma_start(
                    out=outs[g].rearrange("(w p) c -> p w c", p=P),
                    in_=ob[:])
    nc.compile()
    return nc


# ---------------------------------------------------------------- KB: scatter Y
def _build_kb(meta):
    nc = bacc.Bacc("TRN2", target_bir_lowering=False, debug=False,
                   num_swdge_queues=4)
    KL, KH, C = meta["KL"], meta["KH"], meta["C"]
    ncl, nch = meta["ncall_low"], meta["ncall_high"]
    tab = nc.declare_dram_parameter("tab", [NRY, P], bf16, isOutput=False)
    py = nc.declare_dram_parameter("py", [P, W, F], bf16, isOutput=False)
    rsq = nc.declare_dram_parameter("rsq", [P, W], f32, isOutput=False)
    gl = nc.declare_dram_parameter("gl", [P, ncl * 64], i16, isOutput=False)
    gh = nc.declare_dram_parameter("gh", [P, nch * 64], i16, isOutput=False)
    indd = nc.declare_dram_parameter("ind", [P, C * P], fp8, isOutput=False)
    yout = nc.declare_dram_parameter("yout", [R, F], bf16, isOutput=True)

    with tile.TileContext(nc) as tc:
        with (
            tc.tile_pool(name="sb", bufs=1) as sb,
            tc.tile_pool(name="gt", bufs=16) as gt,
            tc.tile_pool(name="indp", bufs=3) as indp,
            tc.tile_pool(name="fl", bufs=2) as fl,
            tc.tile_pool(name="ps", bufs=2, space="PSUM") as ps,
        ):
            gl_t = sb.tile([P, ncl * 64], i16)
            nc.sync.dma_start(out=gl_t[:], in_=gl[:, :])
            gh_t = sb.tile([P, nch * 64], i16)
            nc.sync.dma_start(out=gh_t[:], in_=gh[:, :])
            py_t = sb.tile([P, W, F], bf16)
            nc.sync.dma_start(out=py_t[:], in_=py[:, :, :])
            rsq_t = sb.tile([P, W], f32)
            nc.sync.dma_start(out=rsq_t[:], in_=rsq[:, :])
            ob = sb.tile([P, W, F], bf16)

            srcs = {0: (tab[0:HALF, :], gl_t), 1: (tab[HALF:NRY, :], gh_t)}
            call_tiles = {}
            qctr = [0]

            def get_blk(stream, pos):
                call = pos // CALL
                key = (stream, call)
                if key not in call_tiles:
                    src_ap, idx_t = srcs[stream]
                    g = gt.tile([P, CALL, P], bf16, tag="g")
                    nc.gpsimd.dma_gather(
                        out_ap=g[:], in_ap=src_ap,
                        idxs_ap=idx_t[:, call * 64:(call + 1) * 64],
                        num_idxs=CALL * CHUNK, num_idxs_reg=CALL * CHUNK,
                        elem_size=P, queue_num=qctr[0] % 4)
                    qctr[0] += 1
                    call_tiles[key] = g
                return call_tiles[key], pos % CALL

            Kmax = int((KL + KH).max())
            ci = 0
            pl = ph = 0
            for wi in range(W):
                Kw = int(KL[wi] + KH[wi])
                ind_w = indp.tile([P, Kmax, P], fp8, tag="ind")
                nc.sync.dma_start(out=ind_w[:, 0:Kw, :],
                                  in_=indd[:, ci * P:(ci + Kw) * P])
                pm = ps.tile([P, F], f32, tag="pm")
                for c in range(Kw):
                    if c < KL[wi]:
                        gtile, blk = get_blk(0, pl)
                        pl += 1
                    else:
                        gtile, blk = get_blk(1, ph)
                        ph += 1
                    nc.tensor.matmul(pm[:], lhsT=ind_w[:, c, :],
                                     rhs=gtile[:, blk, F:P],
                                     start=(c == 0), stop=(c == Kw - 1))
                    ci += 1
                nc.vector.scalar_tensor_tensor(
                    out=ob[:, wi, :], in0=pm[:], scalar=rsq_t[:, wi:wi + 1],
                    in1=py_t[:, wi, :], op0=mybir.AluOpType.mult,
                    op1=mybir.AluOpType.add)
            nc.sync.dma_start(out=yout.rearrange("(w p) c -> p w c", p=P),
                              in_=ob[:])
    nc.compile()
    return nc


# ---------------------------------------------------------------- KC: X + dot
def _build_kc(meta):
    nc = bacc.Bacc("TRN2", target_bir_lowering=False, debug=False,
                   num_swdge_queues=4)
    KL, KH, C = meta["KL"], meta["KH"], meta["C"]
    ncl, nch = meta["ncall_low"], meta["ncall_high"]
    tab = nc.declare_dram_parameter("tab", [NRY, P], bf16, isOutput=False)
    px = nc.declare_dram_parameter("px", [P, W, F], bf16, isOutput=False)
    rsq = nc.declare_dram_parameter("rsq", [P, W], f32, isOutput=False)
    gl = nc.declare_dram_parameter("gl", [P, ncl * 64], i16, isOutput=False)
    gh = nc.declare_dram_parameter("gh", [P, nch * 64], i16, isOutput=False)
    indd = nc.declare_dram_parameter("ind", [P, C * P], fp8, isOutput=False)
    indtd = nc.declare_dram_parameter("ind_t", [P, C * P], fp8, isOutput=False)
    abc = nc.declare_dram_parameter("abc", [P, 3 * F + 1], f32, isOutput=False)
    outv = nc.declare_dram_parameter("outv", [P, C], f32, isOutput=True)
    rout = nc.declare_dram_parameter("rout", [P, W], f32, isOutput=True)

    with tile.TileContext(nc) as tc:
        with (
            tc.tile_pool(name="sb", bufs=1) as sb,
            tc.tile_pool(name="gt", bufs=30) as gt,
            tc.tile_pool(name="indp", bufs=3) as indp,
            tc.tile_pool(name="intp", bufs=3) as intp,
            tc.tile_pool(name="fl", bufs=3) as fl,
            tc.tile_pool(name="jk", bufs=4) as jkp,
            tc.tile_pool(name="ps", bufs=2, space="PSUM") as ps,
            tc.tile_pool(name="ps2", bufs=4, space="PSUM") as ps2,
        ):
            gl_t = sb.tile([P, ncl * 64], i16)
            nc.sync.dma_start(out=gl_t[:], in_=gl[:, :])
            gh_t = sb.tile([P, nch * 64], i16)
            nc.sync.dma_start(out=gh_t[:], in_=gh[:, :])
            px_t = sb.tile([P, W, F], bf16)
            nc.sync.dma_start(out=px_t[:], in_=px[:, :, :])
            rsq_t = sb.tile([P, W], f32)
            nc.sync.dma_start(out=rsq_t[:], in_=rsq[:, :])
            abc_t = sb.tile([P, 3 * F + 1], f32)
            nc.sync.dma_start(out=abc_t[:], in_=abc[:, :])
            At, Bt, Ct = (abc_t[:, 0:F], abc_t[:, F:2 * F], abc_t[:, 2 * F:3 * F])
            ot = sb.tile([P, C], f32)
            rb = sb.tile([P, W], f32)

            srcs = {0: (tab[0:HALF, :], gl_t), 1: (tab[HALF:NRY, :], gh_t)}
            call_tiles = {}
            qctr = [0]

            def get_blk(stream, pos):
                call = pos // CALL
                key = (stream, call)
                if key not in call_tiles:
                    src_ap, idx_t = srcs[stream]
                    g = gt.tile([P, CALL, P], bf16, tag="g")
                    nc.gpsimd.dma_gather(
                        out_ap=g[:], in_ap=src_ap,
                        idxs_ap=idx_t[:, call * 64:(call + 1) * 64],
                        num_idxs=CALL * CHUNK, num_idxs_reg=CALL * CHUNK,
                        elem_size=P, queue_num=qctr[0] % 4)
                    qctr[0] += 1
                    call_tiles[key] = g
                return call_tiles[key], pos % CALL

            Kmax = int((KL + KH).max())
            pl = ph = 0
            wstart = np.concatenate([[0], np.cumsum(KL + KH)]).astype(int)
            pend = {}  # wi -> (pm, indt_w, blks)

            def do_select(wi):
                pm, indt_w, blks = pend.pop(wi)
                ci0 = wstart[wi]
                Xw = fl.tile([P, F], f32, tag="X")
                nc.vector.scalar_tensor_tensor(
                    out=Xw[:], in0=pm[:], scalar=rsq_t[:, wi:wi + 1],
                    in1=px_t[:, wi, :], op0=mybir.AluOpType.mult,
                    op1=mybir.AluOpType.add)
                xa = fl.tile([P, F], bf16, tag="xa")
                xt = fl.tile([P, F], f32, tag="xt")
                nc.vector.tensor_tensor(out=xt[:], in0=Xw[:], in1=At,
                                        op=mybir.AluOpType.mult)
                nc.vector.tensor_tensor(out=xa[:], in0=xt[:], in1=Ct,
                                        op=mybir.AluOpType.add)
                jk0 = jkp.tile([P, F], f32, tag="jk0")
                nc.vector.scalar_tensor_tensor(
                    out=jk0[:], in0=Xw[:], scalar=1.0, in1=Bt,
                    op0=mybir.AluOpType.mult, op1=mybir.AluOpType.mult,
                    accum_out=rb[:, wi:wi + 1])
                for c, (gtile, blk) in enumerate(blks):
                    sel = ps2.tile([P, F], f32, tag="sel")
                    nc.tensor.matmul(sel[:], lhsT=indt_w[:, c, :],
                                     rhs=xa[:], start=True, stop=True)
                    jk = jkp.tile([P, F], f32, tag="jk")
                    nc.vector.scalar_tensor_tensor(
                        out=jk[:], in0=sel[:], scalar=1.0,
                        in1=gtile[:, blk, F:P],
                        op0=mybir.AluOpType.mult, op1=mybir.AluOpType.mult,
                        accum_out=ot[:, ci0 + c:ci0 + c + 1])

            for wi in range(W):
                Kw = int(KL[wi] + KH[wi])
                ci = wstart[wi]
                ind_w = indp.tile([P, Kmax, P], fp8, tag="ind")
                nc.sync.dma_start(out=ind_w[:, 0:Kw, :],
                                  in_=indd[:, ci * P:(ci + Kw) * P])
                indt_w = intp.tile([P, Kmax, P], fp8, tag="indt")
                nc.scalar.dma_start(out=indt_w[:, 0:Kw, :],
                                    in_=indtd[:, ci * P:(ci + Kw) * P])
                pm = ps.tile([P, F], f32, tag="pm")
                blks = []
                for c in range(Kw):
                    if c < KL[wi]:
                        gtile, blk = get_blk(0, pl)
                        pl += 1
                    else:
                        gtile, blk = get_blk(1, ph)
                        ph += 1
                    blks.append((gtile, blk))
                    nc.tensor.matmul(pm[:], lhsT=ind_w[:, c, :],
                                     rhs=gtile[:, blk, 0:F],
                                     start=(c == 0), stop=(c == Kw - 1))
                pend[wi] = (pm, indt_w, blks)
                if wi >= 1:
                    do_select(wi - 1)
            do_select(W - 1)
            nc.sync.dma_start(out=outv[:, :], in_=ot[:])
            nc.sync.dma_start(out=rout[:, :], in_=rb[:])
    nc.compile()
    return nc


# ---------------------------------------------------------------- entry point
def kernel(feats, ifeats, keys, ikeys, values, scale, idxs):
    feats = np.asarray(feats, np.float32)
    ifeats = np.asarray(ifeats, np.float32)
    keys = np.asarray(keys, np.float32)
    ikeys = np.asarray(ikeys, np.float32)
    values = np.asarray(values, np.float32)
    scale = np.asarray(scale, np.float32)
    i0 = np.asarray(idxs[0]).astype(np.int64).astype(np.int32)
    i1 = np.asarray(idxs[1]).astype(np.int64).astype(np.int32)
    N = len(i0)
    LAST_HW_NS.clear()

    # ---- host: codebook boundary metadata (md = [m(15) | d(15) | base])
    def md_cols(k_raw, axis, off):
        tk = 1.0 / (1.0 + np.exp(-np.sort(k_raw[axis], axis=-1)))  # (F, NQ)
        m = 0.5 * (tk[:, :NB] + tk[:, 1:])
        d = tk[:, 1:] - tk[:, :NB]
        base = tk[:, 0:1] + off
        return np.concatenate([m, d, base], axis=1)  # (F, 31)

    mdX = np.concatenate([md_cols(keys, 0, 0.0), md_cols(ikeys, 0, -0.5)],
                         axis=0).astype(np.float32)      # (128, 31) px|ipx
    mdY = np.concatenate([md_cols(keys, 1, 0.0), md_cols(ikeys, 1, -0.5)],
                         axis=0).astype(np.float32)      # (128, 31) py|ipy

    # ---- host: K1 input shards, features on partitions (top=feats, bot=ifeats)
    def shard_T(base, k):
        lo = base + k * R
        hi = min(base + D0, lo + R)
        out = np.zeros((P, R), np.float32)
        n = hi - lo
        if n > 0:
            out[0:F, 0:n] = feats[lo:hi].T
            out[F:P, 0:n] = ifeats[lo:hi].T
        return out

    nc1 = _build_k1()
    maps1 = []
    for k in range(NC):
        maps1.append({
            "inX": shard_T(0, k), "mdX": mdX,
            "inY": shard_T(D0, k), "mdY": mdY,
        })
    r1 = _run(nc1, maps1)

    # outX rows: [px | ipx]; outY rows: [py | ipy]  (R, 128) per core
    tabB = np.concatenate([r1[k]["outX"] for k in range(NC)], axis=0)
    outY = [r1[k]["outY"] for k in range(NC)]

    # ---- host: edge packing + count metadata
    cnt0 = np.bincount(i0, minlength=NRY).astype(np.float64)
    cnt1 = np.bincount(i1, minlength=NRY).astype(np.float64)
    rsq0 = (1.0 / np.sqrt(cnt0 + 1e-12)).astype(np.float32)
    rsq1 = (1.0 / np.sqrt(cnt1 + 1e-12)).astype(np.float32)

    def rsq_shard(rsq_full, k):
        return rsq_full[k * R:(k + 1) * R].reshape(W, P).T.copy()  # (P, W)

    coresB, metaB = _pack_pass(i1, i0, want_sel=False)  # dest=i1, gather i0
    coresC, metaC = _pack_pass(i0, i1, want_sel=True)   # dest=i0, gather i1

    # ---- KB
    ncb = _build_kb(metaB)
    maps2 = []
    for k in range(NC):
        py_k = outY[k][:, 0:F].reshape(W, P, F).transpose(1, 0, 2).copy()
        maps2.append({
            "tab": tabB,
            "py": py_k,
            "rsq": rsq_shard(rsq1, k),
            "gl": coresB[k]["gl"], "gh": coresB[k]["gh"],
            "ind": coresB[k]["ind"],
        })
    r2 = _run(ncb, maps2)

    # tabC rows: [ipy | Y]
    y_full = np.concatenate([r2[k]["yout"] for k in range(NC)], axis=0)
    ipy_full = np.concatenate([outY[k][:, F:P] for k in range(NC)], axis=0)
    tabC = np.concatenate([ipy_full, y_full], axis=1)  # (NRY, 128)

    # ---- host: folded values/scale constants
    s = float(scale[0])
    V = values[0]  # (4, F)
    Arow = s * (V[0] - V[1] - V[2] + V[3])
    Brow = s * (V[1] - V[3])
    Crow = s * (V[2] - V[3])
    sumD = s * float(V[3].sum())
    abc = np.concatenate([
        np.tile(Arow, (P, 1)), np.tile(Brow, (P, 1)), np.tile(Crow, (P, 1)),
        np.full((P, 1), sumD, np.float32)], axis=1).astype(np.float32)

    # ---- KC
    ncc = _build_kc(metaC)
    maps3 = []
    for k in range(NC):
        px_k = r1[k]["outX"][:, 0:F].reshape(W, P, F).transpose(1, 0, 2).copy()
        maps3.append({
            "tab": tabC,
            "px": px_k,
            "rsq": rsq_shard(rsq0, k),
            "gl": coresC[k]["gl"], "gh": coresC[k]["gh"],
            "ind": coresC[k]["ind"], "ind_t": coresC[k]["ind_t"],
            "abc": abc,
        })
    r3 = _run(ncc, maps3)

    # r_full[d] = B·X[d] (device) + sumD (host)
    r_full = np.concatenate(
        [r3[k]["rout"].T.reshape(R) for k in range(NC)]) + np.float32(sumD)

    out = np.zeros(N, np.float32)
    for k in range(NC):
        vals = r3[k]["outv"]          # (P, C)
        perm = coresC[k]["perm"]      # (C, P)
        m = perm >= 0
        out[perm[m]] = vals.T[m]
    out += r_full[i0]
    return out
